# revision 19
# baseline (speedup 1.0000x reference)
"""Causal self-attention (B=1, T=4096, D=1024, H=16, dh=64) on 8 trn2 NeuronCores.

Tensor-parallel over heads: each core owns 2 of 16 heads and computes a
rank-128 partial of the output projection; the host sums 8 fp16 partials.

Single merged software pipeline per core, iteration c = 0..8:
  [attention qc=c] -> [out-projection of chunk c-1] -> [normalize chunk c]
with QKV+RoPE for chunk c at the top.  Chunk 0 runs fully in bf16 (exact
path protects short-context rows); chunks 1-7 run the QKV projection and
the P@V attention matmul in fp8e4m3 with DoubleRow perf mode (256-deep
contraction at 0.5 cycles/row), which roughly halves PE work.  Causal
masking zeroes masked probabilities post-exp (memset + masked
multiply), never reading PSUM columns the restricted S matmuls skip.  Softmax
denominators ride a ones-column in V; normalization uses a gpsimd
partition-broadcast + fast DVE reciprocal (no DRAM bounce).
"""

import sys

sys.path.insert(0, "/opt/trn_rl_repo")

import numpy as np

import concourse.bass as bass
import concourse.tile as tile
from concourse import bacc, mybir
from concourse.bass_utils import run_bass_kernel_spmd

T = 4096
D = 1024
H = 16
DH = 64
NC = 8
HL = H // NC  # heads per core (2)
DL = HL * DH  # local feature width (128)
NCH = 8  # T-chunks of 512
CW = 512  # chunk width
VB = 160  # v pair block (fp8): [v(kb_even) 64 | 1 | pad 15 | v(kb_odd) 64 | 1 | pad 15]
# sub-block stride 80 B: DoubleRow LDWEIGHTS requires even, 16B-aligned steps
VB0 = 130  # chunk-0 bf16 v block: [v_h0 64 | 1 | v_h1 64 | 1] per kb

F32 = mybir.dt.float32
F32R = mybir.dt.float32r
BF16 = mybir.dt.bfloat16
FP16 = mybir.dt.float16
FP8 = mybir.dt.float8e4


def build_nc():
    nc = bacc.Bacc(
        "TRN2", target_bir_lowering=False, debug=False, num_devices=NC
    )

    # ---- DRAM I/O -------------------------------------------------------
    xT0_d = nc.dram_tensor("xT0", [D, CW], BF16, kind="ExternalInput").ap()
    x8_d = nc.dram_tensor("x8", [128, 8, T], FP8, kind="ExternalInput").ap()
    wqkvT_d = nc.dram_tensor("wqkvT", [D, 3 * DL], BF16, kind="ExternalInput").ap()
    wqkv8_d = nc.dram_tensor("wqkv8", [128, 4, 2, 3 * DL], FP8, kind="ExternalInput").ap()
    woutT_d = nc.dram_tensor("woutT", [DL, D], BF16, kind="ExternalInput").ap()
    cos2_d = nc.dram_tensor("cos2", [DL, T], F32R, kind="ExternalInput").ap()
    sin2_d = nc.dram_tensor("sin2", [DL, T], F32R, kind="ExternalInput").ap()
    p128_d = nc.dram_tensor("p128", [DL, DL], F32R, kind="ExternalInput").ap()
    ident_d = nc.dram_tensor("ident", [128, 128], BF16, kind="ExternalInput").ap()
    # multiplicative masks [128, 4*512]: mask_j[k, q] = 1 iff q >= j*128 + k
    dmask_d = nc.dram_tensor("dmask", [128, 4 * 512], BF16, kind="ExternalInput").ap()
    out_d = nc.dram_tensor("outp", [T, D], FP16, kind="ExternalOutput").ap()

    with tile.TileContext(nc) as tc:
        with tc.tile_pool(name="consts", bufs=1) as cpool, \
             tc.tile_pool(name="persist", bufs=1) as ppool:
            # ---- constants needed immediately (chunk-0 weights + x) ----
            wt = cpool.tile([128, 8, 3 * DL], BF16, tag="wt")
            nc.sync.dma_start(
                out=wt[:], in_=wqkvT_d.rearrange("(d p) f -> p d f", p=128)
            )
            p128 = cpool.tile([DL, DL], F32R, tag="p128")
            nc.gpsimd.dma_start(out=p128[:], in_=p128_d)
            ident = cpool.tile([128, 128], BF16, tag="ident")
            nc.gpsimd.dma_start(out=ident[:], in_=ident_d)

            # ---- persistent activations --------------------------------
            qT = ppool.tile([DL, T], BF16, tag="qT")
            kT = ppool.tile([DL, T], BF16, tag="kT")
            # fp8 v, natural layout, one tile per head: 16 pairs x 130
            vn8 = [ppool.tile([128, 16 * VB], FP8, tag=f"vn8_{h}", name=f"vn8_{h}")
                   for h in range(HL)]
            # chunk-0 bf16 v (4 kb blocks x [v_h0|1|v_h1|1])
            vn0 = ppool.tile([128, 4 * VB0], BF16, tag="vn0")
            attn_n = ppool.tile([DL, T], BF16, tag="attn_n")

            # ones columns: memset whole v tiles to 1.0; later copies
            # overwrite the 64-wide value blocks, leaving the ones columns.
            for h in range(HL):
                nc.gpsimd.memset(vn8[h][:], 1.0)
            nc.gpsimd.memset(vn0[:], 1.0)

            # ---- deferred constants (other queues / behind startup) ----
            wt8 = cpool.tile([128, 4, 2, 3 * DL], FP8, tag="wt8")
            nc.sync.dma_start(out=wt8[:], in_=wqkv8_d)
            cos2 = cpool.tile([DL, T], F32R, tag="cos2")
            sin2 = cpool.tile([DL, T], F32R, tag="sin2")
            woutT = cpool.tile([DL, D], BF16, tag="woutT")
            dmask = cpool.tile([128, 4 * 512], BF16, tag="dmask")
            nc.gpsimd.dma_start(out=cos2[:], in_=cos2_d)
            nc.gpsimd.dma_start(out=sin2[:], in_=sin2_d)
            nc.gpsimd.dma_start(out=dmask[:], in_=dmask_d)
            nc.gpsimd.dma_start(out=woutT[:], in_=woutT_d)

            # ones row used to broadcast the softmax sums across partitions
            # via the PE (out[i, q] = sum_p ones[p, i] * ev[p, q], contraction
            # over the single partition holding the sums row)
            ones_f32 = cpool.tile([128, 128], F32, tag="ones_f32")
            nc.gpsimd.memset(ones_f32[:], 1.0)
            ones_bc = cpool.tile([128, 128], F32R, tag="ones_bc")
            nc.vector.tensor_copy(ones_bc[:], ones_f32[:])

            with tc.tile_pool(name="xp", bufs=2) as xpool, \
                 tc.tile_pool(name="tmpa", bufs=3) as tpool, \
                 tc.tile_pool(name="ptf8", bufs=4) as pt8pool, \
                 tc.tile_pool(name="ptbf", bufs=2) as ptbpool, \
                 tc.tile_pool(name="ptsc", bufs=3) as ptspool, \
                 tc.tile_pool(name="evp", bufs=3) as evpool, \
                 tc.tile_pool(name="nrm", bufs=2) as npool, \
                 tc.tile_pool(name="op", bufs=3) as opool, \
                 tc.tile_pool(name="psX", bufs=2, space="PSUM") as psX, \
                 tc.tile_pool(name="psST", bufs=1, space="PSUM") as psST, \
                 tc.tile_pool(name="psAT", bufs=1, space="PSUM") as psAT:

                # x chunk prefetch (one descriptor per chunk)
                xts = {}

                def fetch_chunk(ci):
                    if ci == 0:
                        xt = xpool.tile([128, 8, CW], BF16, tag="xt0")
                        nc.sync.dma_start(
                            out=xt[:],
                            in_=xT0_d.rearrange("(d p) t -> p d t", p=128),
                        )
                    else:
                        xt = xpool.tile([128, 8, CW], FP8, tag="xt8")
                        nc.sync.dma_start(
                            out=xt[:], in_=x8_d[:, :, ci * CW:(ci + 1) * CW]
                        )
                    xts[ci] = xt

                def do_qkv(cc, xt):
                    s = cc * CW

                    def qkv_matmuls(pp, idx):
                        if cc == 0:
                            for d in range(8):
                                nc.tensor.matmul(
                                    pp[:],
                                    lhsT=wt[:, d, idx * DL:(idx + 1) * DL],
                                    rhs=xt[:, d, :],
                                    start=(d == 0), stop=(d == 7),
                                )
                        else:
                            for j in range(4):
                                nc.tensor.matmul(
                                    pp[:],
                                    lhsT=wt8[:, j, :, idx * DL:(idx + 1) * DL],
                                    rhs=xt[:, 2 * j:2 * j + 2, :],
                                    start=(j == 0), stop=(j == 3),
                                    perf_mode=mybir.MatmulPerfMode.DoubleRow,
                                )

                    for idx, dst in ((0, qT), (1, kT)):
                        pp = psX.tile([128, CW], F32, tag="ppx")
                        qkv_matmuls(pp, idx)
                        praw = tpool.tile([128, CW], F32R, tag="praw")
                        nc.vector.tensor_copy(praw[:], pp[:])
                        rot = psX.tile([128, CW], F32, tag="ppx")
                        nc.tensor.matmul(
                            rot[:], lhsT=p128[:], rhs=praw[:],
                            start=True, stop=True,
                        )
                        dstv = dst[:, s:s + CW]
                        nc.vector.tensor_mul(dstv, praw[:], cos2[:, s:s + CW])
                        rtmp = tpool.tile([128, CW], BF16, tag="rtmp")
                        nc.vector.tensor_mul(rtmp[:], rot[:], sin2[:, s:s + CW])
                        nc.gpsimd.tensor_add(dstv, dstv, rtmp[:])

                    # v: compute vT, PE-transpose to natural, store fp8
                    vp = psX.tile([128, CW], F32, tag="ppx")
                    qkv_matmuls(vp, 2)
                    vtmp = tpool.tile([128, CW], BF16, tag="vtmp")
                    nc.vector.tensor_copy(vtmp[:], vp[:])
                    for b in range(4):
                        kb = 4 * cc + b
                        pr, sb = kb // 2, kb % 2
                        tpx = psX.tile([128, 512], F32, tag="ppx")
                        tp = tpx.bitcast(BF16)[:, 0:128]
                        nc.tensor.transpose(
                            tp, vtmp[:, b * 128:(b + 1) * 128], ident[:]
                        )
                        for h in range(HL):
                            o8 = pr * VB + sb * 80
                            nc.vector.tensor_copy(
                                vn8[h][:, o8:o8 + 64],
                                tp[:, h * DH:(h + 1) * DH],
                            )
                        if cc == 0:
                            o0 = kb * VB0
                            nc.vector.tensor_copy(
                                vn0[:, o0:o0 + 64], tp[:, 0:DH])
                            nc.vector.tensor_copy(
                                vn0[:, o0 + 65:o0 + 129], tp[:, DH:128])

                def do_attention(qc):
                    kmax = 4 * (qc + 1)
                    q0 = qc * CW
                    ats = [psAT.tile([DH + 1, CW], F32, tag=f"at{h}", name=f"at{h}")
                           for h in range(HL)]
                    sts = [psST.tile([128, 1024], F32, tag=f"st{h}", name=f"st{h}")
                           for h in range(HL)]

                    # exp into pt; diag blocks: memset-0 the masked prefix,
                    # exp only the S-written range, then multiply by the
                    # causal mask post-exp (off the PSUM path)
                    def fill_pt(half, h, kb, i):
                        j = kb - 4 * qc
                        src = sts[h]
                        if j < 0:
                            nc.scalar.activation(
                                half,
                                src[:, i * 512:(i + 1) * 512],
                                mybir.ActivationFunctionType.Exp,
                                scale=0.125,
                            )
                            return
                        qo = 128 * j
                        if qo:
                            nc.gpsimd.memset(half[:, 0:qo], 0.0)
                        ptb = ptspool.tile([128, 512], BF16, tag="ptb")
                        nc.scalar.activation(
                            ptb[:, 0:512 - qo],
                            src[:, i * 512 + qo:(i + 1) * 512],
                            mybir.ActivationFunctionType.Exp,
                            scale=0.125,
                        )
                        nc.vector.tensor_mul(
                            half[:, qo:512],
                            ptb[:, 0:512 - qo],
                            dmask[:, j * 512 + qo:(j + 1) * 512],
                        )

                    npairs = kmax // 2
                    for pr in range(npairs):
                        kbs = (2 * pr, 2 * pr + 1)
                        # S matmuls: heads interleaved -> PE row groups
                        for i, kb in enumerate(kbs):
                            j = kb - 4 * qc  # diag block index (>=0 if diag)
                            qoff = 128 * j if j >= 0 else 0
                            for h in range(HL):
                                hs = h * DH
                                nc.tensor.matmul(
                                    sts[h][:, i * 512 + qoff:(i + 1) * 512],
                                    lhsT=kT[hs:hs + DH, kb * 128:(kb + 1) * 128],
                                    rhs=qT[hs:hs + DH, q0 + qoff:q0 + CW],
                                    start=True, stop=True,
                                )
                        if qc == 0:
                            # bf16 path: per-kb AV with ones col
                            pts = []
                            for h in range(HL):
                                pt = ptbpool.tile([128, 1024], BF16, tag=f"ptb{h}")
                                for i, kb in enumerate(kbs):
                                    fill_pt(pt[:, i * 512:(i + 1) * 512], h, kb, i)
                                pts.append(pt)
                            for i, kb in enumerate(kbs):
                                for h in range(HL):
                                    o = kb * VB0 + h * 65
                                    nc.tensor.matmul(
                                        ats[h][:],
                                        lhsT=vn0[:, o:o + 65],
                                        rhs=pts[h][:, i * 512:(i + 1) * 512],
                                        start=(kb == 0), stop=(kb == kmax - 1),
                                        skip_group_check=True,
                                    )
                        else:
                            # fp8 path: DoubleRow AV per pair
                            for h in range(HL):
                                pt = pt8pool.tile([128, 2, 512], FP8, tag=f"pt8{h}")
                                for i, kb in enumerate(kbs):
                                    fill_pt(pt[:, i, :], h, kb, i)
                                v4 = vn8[h][:].rearrange(
                                    "p (r s d) -> p r s d", s=2, d=80)
                                nc.tensor.matmul(
                                    ats[h][:],
                                    lhsT=v4[:, pr, :, 0:65],
                                    rhs=pt[:],
                                    start=(pr == 0), stop=(pr == npairs - 1),
                                    perf_mode=mybir.MatmulPerfMode.DoubleRow,
                                    skip_group_check=True,
                                )
                    return ats

                def do_ev(ats):
                    evs = []
                    for h in range(HL):
                        ev = evpool.tile([DH + 1, CW], F32R, tag=f"ev{h}")
                        nc.vector.tensor_copy(ev[:], ats[h][:])
                        evs.append(ev)
                    return evs

                def do_outproj(pc):
                    for tbl in range(4):
                        tb = pc * 4 + tbl
                        osb = opool.tile([128, D], FP16, tag="osb")
                        for ec in range(2):
                            op = psX.tile([128, 512], F32, tag="ppx")
                            nc.tensor.matmul(
                                op[:],
                                lhsT=attn_n[:, tb * 128:(tb + 1) * 128],
                                rhs=woutT[:, ec * 512:(ec + 1) * 512],
                                start=True, stop=True,
                            )
                            nc.vector.tensor_copy(
                                osb[:, ec * 512:(ec + 1) * 512], op[:])
                        nc.gpsimd.dma_start(
                            out=out_d[tb * 128:(tb + 1) * 128, :], in_=osb[:]
                        )

                def do_normalize(qc, evs):
                    q0 = qc * CW
                    for h in range(HL):
                        # broadcast the sums row across partitions on the PE
                        rbp = psX.tile([128, CW], F32, tag="ppx")
                        nc.tensor.matmul(
                            rbp[:],
                            lhsT=ones_bc[DH:DH + 1, :],
                            rhs=evs[h][DH:DH + 1, :],
                            start=True, stop=True,
                        )
                        rcp = npool.tile([DH, CW], F32, tag=f"rcp{h}",
                                         name=f"rcp{h}")
                        nc.vector.reciprocal_approx_fast(
                            rcp[:], rbp[0:DH, :])
                        if h == 0:
                            nc.vector.tensor_mul(
                                attn_n[0:DH, q0:q0 + CW],
                                evs[0][0:DH, :], rcp[:],
                            )
                        else:
                            # normalize in partitions 0-63, DMA to rows 64+
                            n1 = npool.tile([DH, CW], BF16, tag="n1")
                            nc.vector.tensor_mul(
                                n1[:], evs[1][0:DH, :], rcp[:])
                            nc.gpsimd.dma_start(
                                out=attn_n[DH:2 * DH, q0:q0 + CW], in_=n1[:]
                            )

                fetch_chunk(0)
                fetch_chunk(1)

                evs_cur = None
                for c in range(10):
                    if c < 8:
                        do_qkv(c, xts.pop(c))
                        if c + 2 < 8:
                            fetch_chunk(c + 2)
                    if 1 <= c <= 8:
                        ats_cur = do_attention(c - 1)
                        evs_cur = do_ev(ats_cur)
                    if c >= 2:
                        do_outproj(c - 2)
                    if 1 <= c <= 8:
                        do_normalize(c - 1, evs_cur)

    nc.compile()
    return nc


def _round_f32r(a):
    """Round fp32 array to the fp32r format (12-bit mantissa, RNE-ish)."""
    b = np.ascontiguousarray(a, np.float32).view(np.uint32)
    b = ((b + 0x800) & np.uint32(0xFFFFF000)).astype(np.uint32)
    return b.view(np.float32)


def _host_constants():
    import ml_dtypes
    inv_freq = 1.0 / (10000.0 ** (np.arange(0, DH, 2, dtype=np.float64) / DH))
    t = np.arange(T, dtype=np.float64)
    freqs = np.outer(t, inv_freq)  # [T, 32]
    emb = np.concatenate([freqs, freqs], axis=-1)  # [T, 64]
    cos = np.cos(emb).astype(np.float32).T  # [64, T]
    sin = np.sin(emb).astype(np.float32).T  # [64, T]
    sinS = sin.copy()
    sinS[0:DH // 2] *= -1.0  # fold rotate_half's negation into the table
    cos2 = _round_f32r(np.ascontiguousarray(np.tile(cos, (HL, 1))))  # [128, T]
    sin2 = _round_f32r(np.ascontiguousarray(np.tile(sinS, (HL, 1))))

    # swap-halves permutation (per 64-row head block), symmetric
    p1 = np.zeros((DH, DH), np.float32)
    half = DH // 2
    p1[np.arange(half), np.arange(half) + half] = 1.0
    p1[np.arange(half) + half, np.arange(half)] = 1.0
    p128 = np.block([
        [p1, np.zeros((DH, DH), np.float32)],
        [np.zeros((DH, DH), np.float32), p1],
    ]).astype(np.float32)

    ident = np.eye(128, dtype=np.float32).astype(ml_dtypes.bfloat16)

    # multiplicative diag masks [128, 4, 512]: 1 iff q >= j*128 + k
    dmask = np.zeros((128, 4, 512), np.float32)
    kk = np.arange(128)[:, None]
    qq = np.arange(512)[None, :]
    for j in range(4):
        dmask[:, j, :] = (qq >= j * 128 + kk).astype(np.float32)
    dmask = np.ascontiguousarray(
        dmask.reshape(128, 4 * 512)).astype(ml_dtypes.bfloat16)
    return cos2, sin2, p128, ident, dmask


_NC_CACHE = None


def _get_nc():
    global _NC_CACHE
    if _NC_CACHE is None:
        _NC_CACHE = build_nc()
    return _NC_CACHE


def _in_maps(x, W_qkv, W_out):
    import ml_dtypes
    FP8NP = ml_dtypes.float8_e4m3
    x2 = np.asarray(x, np.float32).reshape(T, D)
    W_qkv = np.asarray(W_qkv, np.float32)
    W_out = np.asarray(W_out, np.float32)
    xT = np.ascontiguousarray(x2.T)  # [1024, 4096] fp32
    xT0 = xT[:, 0:CW].astype(ml_dtypes.bfloat16)
    x8 = np.ascontiguousarray(
        xT.reshape(8, 128, T).transpose(1, 0, 2)).astype(FP8NP)  # [128,8,T]
    cos2, sin2, p128, ident, dmask = _host_constants()

    Wq, Wk, Wv = W_qkv[0:D], W_qkv[D:2 * D], W_qkv[2 * D:3 * D]
    in_maps = []
    for cid in range(NC):
        h0, h1 = HL * cid, HL * cid + 1
        rows = []
        for Wp in (Wq, Wk, Wv):
            rows.append(Wp[h0 * DH:(h0 + 1) * DH])
            rows.append(Wp[h1 * DH:(h1 + 1) * DH])
        wfull = np.ascontiguousarray(np.concatenate(rows, axis=0).T)  # [1024, 384]
        wqkvT = wfull.astype(ml_dtypes.bfloat16)
        wqkv8 = np.ascontiguousarray(
            wfull.reshape(4, 2, 128, 3 * DL).transpose(2, 0, 1, 3)
        ).astype(FP8NP)  # [128, 4, 2, 384]
        cols = np.r_[h0 * DH:(h0 + 1) * DH, h1 * DH:(h1 + 1) * DH]
        woutT = np.ascontiguousarray(
            W_out[:, cols].T).astype(ml_dtypes.bfloat16)  # [128, D]
        in_maps.append({
            "xT0": xT0, "x8": x8, "wqkvT": wqkvT, "wqkv8": wqkv8,
            "woutT": woutT, "cos2": cos2, "sin2": sin2, "p128": p128,
            "ident": ident, "dmask": dmask,
        })
    return in_maps


def _run(x, W_qkv, W_out, **spmd_kwargs):
    nc = _get_nc()
    res = run_bass_kernel_spmd(
        nc, _in_maps(x, W_qkv, W_out), core_ids=list(range(NC)), **spmd_kwargs
    )
    out = res.results[0]["outp"].astype(np.float64)
    for cid in range(1, NC):
        out += res.results[cid]["outp"]
    return out.astype(np.float32).reshape(1, T, D), res


def kernel(x, W_qkv, W_out):
    out, _ = _run(x, W_qkv, W_out)
    return out
